# revision 1
# baseline (speedup 1.0000x reference)
"""Trainium2 Bass kernel for nn_Encoder_29661044146233 (gnn_message_passing).

Approach
--------
The whole network is linear per output frame, so (as in the earlier
version) it folds into a single 22-tap stride-8 conv (88 -> 66 channels)
whose weights are probed on the host in float64.  This version restructures
the device matmuls around 8-frame *input blocks* (704 values, zero-padded
to 768 = 6*128) so the contraction tiles the full 128-partition dim:

    out[t] = A xblk[t-1] + B xblk[t] + C xblk[t+1] + bias

A/B/C are [66, 768] (taps 0-6 / 7-14 / 15-21).  The three maps are
M-packed into two weight groups G1 = [B; A[0:62]] (128 rows) and
G2 = [A[62:66]; C] (70 rows), each computed with 6 accumulating K=128
matmuls per 2-batch pair (N = 2*256) -> 12 matmul-equivalents per batch
at N=256 instead of the previous 22.  The bias rides in a spare weight
column against a constant-1 input slot.  DVE assembles the shifted sum
out[t] = zB[t] + zA[t-1] + zC[t+1] from the two PSUM tiles; boundary
columns t=0/255 get the probed edge-delta correction (same as before).

Inputs/weights go to the device in float16 (10-bit mantissa; measured
harmless at this tolerance), halving HBM traffic; PSUM accumulates fp32.
Few, large DMAs (2-batch input chunks, single weight blobs, 2 output
stores) keep the shared HWDGE/DMA-engine devices off the critical path.
"""

import os
import sys

for _p in ("/opt/trn_rl_repo", "/root/.axon_site/_ro/trn_rl_repo"):
    if os.path.isdir(_p) and _p not in sys.path:
        sys.path.append(_p)

import numpy as np

TOPOLOGY = [0, 0, 1, 2, 3, 4, 0, 6, 7, 8, 0, 10, 11, 12, 12, 14, 15, 16, 12, 18, 19, 20]
J = 22
POS, OFF = 3, 1
CIN = 88
COUT = 66
NTAP = 22
NEDGE = 15
B, F, T = 128, 2048, 256
NCORES = 8
BL = B // NCORES          # batch per core
NPAIR = BL // 2
UB = 258                  # blocks incl one zero pad each side
BK = 768                  # padded block length (704 data + 1 bias + 63 pad)
KC = 6                    # K chunks of 128 per block
NCHUNK = 16               # B: 6 chunks, A: 5 (head rows ride the pad), C: 5
XC = UB * KC              # sbuf cols per batch
# (group slot base, rhs block window start, chunk indices):
#   out[t] = zB[t] + zA[t-1] + zC[t+1] as rhs-shifted accumulation groups.
#   C's chunk 5 is identically zero (taps 15-21 end at block col 616); A's
#   chunk-0 rows (block cols 88..128) are duplicated into the pad rows
#   705..745 so A's group needs only chunks 1-5 — 16 matmuls total, the
#   ceil(1937/128) K-packing floor.
GROUPS = [(0, 1, (0, 1, 2, 3, 4, 5)),
          (6, 0, (1, 2, 3, 4, 5)),
          (11, 2, (0, 1, 2, 3, 4))]


# ---------------------------------------------------------------------------
# host-side weight composition (float64 impulse probing) — unchanged
# ---------------------------------------------------------------------------

def _adj():
    a = np.zeros((J, J), np.float64)
    for i, p in enumerate(TOPOLOGY):
        if i:
            a[p, i] = 1.0
    return a


def _conv_np(z, w, b):
    Bn, Fn, C = z.shape
    zp = np.zeros((Bn, Fn + 2, C), z.dtype)
    zp[:, 1:Fn + 1] = z
    Fo = Fn // 2
    out = np.zeros((Bn, Fo, w.shape[0]), z.dtype)
    for k in range(4):
        out += zp[:, k:k + 2 * Fo:2] @ w[:, :, k].T
    return out + b


def _graph_mat(A, n2n_w, n2n_b, e2n_we, e2n_wn, e2n_b,
               n2e_wn, n2e_we, n2e_b, lin_w, lin_b):
    def apply(z):
        sh = z.shape[:-1]
        zz = z.reshape(-1, J, 4)
        node, edge = zz[..., :POS], zz[..., POS:]
        agg_n = np.einsum('ij,bjc->bic', A, node)
        agg_e = np.einsum('ij,bjc->bic', A, edge)
        f1 = agg_n @ n2n_w + n2n_b
        f2 = agg_e @ e2n_we + node @ e2n_wn + e2n_b
        new_edge = (np.einsum('ji,bjc->bic', A, node) @ n2e_wn
                    + edge @ n2e_we + n2e_b)
        h = np.concatenate([f1, f2], axis=-1) @ lin_w + lin_b
        return np.concatenate([h, new_edge], axis=-1).reshape(*sh, 88)

    g = apply(np.zeros((1, 88)))[0]
    G = apply(np.eye(88)) - g
    return G.T, g


def _compose(P):
    A = _adj()
    P64 = {k: np.asarray(v, np.float64) for k, v in P.items()}
    gnames = ('n2n_w', 'n2n_b', 'e2n_we', 'e2n_wn', 'e2n_b',
              'n2e_wn', 'n2e_we', 'n2e_b', 'lin_w', 'lin_b')
    G1, g1 = _graph_mat(A, *[P64['g1_' + s] for s in gnames])
    G2, g2 = _graph_mat(A, *[P64['g2_' + s] for s in gnames])
    keep = np.array([4 * j + c for j in range(J) for c in range(POS)])

    def pipeline(x88):
        y = _conv_np(x88, P64['conv1_w'], P64['conv1_b'])
        y = y @ G1.T + g1
        y = _conv_np(y, P64['conv2_w'], P64['conv2_b'])
        y = y @ G2.T + g2
        y = _conv_np(y, P64['conv3_w'], P64['conv3_b'])
        return y[..., keep]

    Fp = 256
    Tp = Fp // 8
    zb = pipeline(np.zeros((1, Fp, 88)))[0]
    bint, bl, br = zb[Tp // 2], zb[0], zb[Tp - 1]

    mid = Fp // 2
    probes = np.zeros((8 * 88, Fp, 88))
    for r in range(8):
        for ic in range(88):
            probes[r * 88 + ic, mid + r, ic] = 1.0
    resp = pipeline(probes) - zb
    wint = np.zeros((NTAP, COUT, CIN))
    for r in range(8):
        for t in range(Tp):
            m = (mid + r) - 8 * t + 7
            if 0 <= m < NTAP:
                wint[m] = resp[r * 88:(r + 1) * 88, t, :].T

    probes = np.zeros((NEDGE * 88, Fp, 88))
    for f in range(NEDGE):
        for ic in range(88):
            probes[f * 88 + ic, f, ic] = 1.0
    resp = pipeline(probes) - zb
    wl = np.stack([resp[f * 88:(f + 1) * 88, 0, :].T for f in range(NEDGE)])

    probes = np.zeros((NEDGE * 88, Fp, 88))
    for f in range(NEDGE):
        for ic in range(88):
            probes[f * 88 + ic, Fp - NEDGE + f, ic] = 1.0
    resp = pipeline(probes) - zb
    wr = np.stack([resp[f * 88:(f + 1) * 88, Tp - 1, :].T for f in range(NEDGE)])

    return dict(wint=wint, bint=bint, wl=wl, wr=wr, bl=bl, br=br)


# ---------------------------------------------------------------------------
# device program (built/compiled once, reused across calls)
# ---------------------------------------------------------------------------

_STATE = {}

NDELTA = 4
CE_W = 2 * NDELTA * COUT
CE_X = 2 * NDELTA * BL


DEFAULT_OPTS = dict(
    warm_n=12,          # warm-up matmul count (bridge p-state ramp to x0)
    assembly="act",     # PSUM->SBUF copy: "split" (ACT+DVE) or "act"
    memset_cols=2,      # zeroed scratch cols (warmups read garbage beyond)
    head_colhalf=False,  # first chain as two N=128 col-halves
    tail_cols=64,       # final col-chain width (T = no split)
    xs_bufs=5,
)


def _build_device(opts=None):
    import concourse.bass as bass  # noqa: F401
    import concourse.tile as tile
    from concourse import bacc, mybir

    o_ = dict(DEFAULT_OPTS)
    if opts:
        o_.update(opts)
    f32 = mybir.dt.float32
    f16 = mybir.dt.float16
    nc = bacc.Bacc("TRN2", target_bir_lowering=False, debug=False,
                   num_devices=NCORES)

    wsb_d = nc.dram_tensor("wsb", [128, NCHUNK * COUT], f16, kind="ExternalInput")
    we_d = nc.dram_tensor("we", [CIN, CE_W + CE_X], f16, kind="ExternalInput")
    xh_d = nc.dram_tensor("xh", [NPAIR, 128, 2, XC], f16, kind="ExternalInput")
    out_d = nc.dram_tensor("out", [COUT, BL, T], f16, kind="ExternalOutput")

    with tile.TileContext(nc) as tc:
        with (
            tc.tile_pool(name="consts", bufs=1) as consts,
            tc.tile_pool(name="xs", bufs=o_["xs_bufs"]) as xspool,
            tc.tile_pool(name="ps1", bufs=4, space="PSUM") as ps1pool,
            tc.tile_pool(name="warm", bufs=1, space="PSUM") as warmpool,
            tc.tile_pool(name="ob", bufs=1) as opool,
        ):
            # PE warm-up: dummy bf16 matmuls on scratch, no DMA deps. Fills
            # the ~3us p-state ramp while the first DMAs stream. Only the
            # first cols are zeroed; reading garbage is fine (never read).
            bf16 = mybir.dt.bfloat16
            scratch = consts.tile([CIN, 162], f32)
            if o_["memset_cols"]:
                nc.vector.memset(scratch[:, 0:o_["memset_cols"]], 0.0)
            s16 = scratch[:].bitcast(bf16)          # [88, 324] bf16 view
            wps = warmpool.tile([COUT, 256], f32)
            for _ in range(o_["warm_n"]):
                nc.tensor.matmul(wps[:], lhsT=s16[:, 0:COUT],
                                 rhs=s16[:, 66:322], start=True, stop=True)

            # DMA order tuned for start latency: head weights, batch-0 input
            # (in column halves), edge blob, rest of the weights, then
            # single-batch streams for batches 1-3 (matching the PE's early
            # consumption), then 2-batch chunks.
            HEAD = 6 * COUT
            XHALF = 131 * KC    # input cols covering out cols [0, 128)
            wsb = consts.tile([128, NCHUNK * COUT], f16)
            wv = wsb[:].rearrange("p (k m) -> p k m", k=NCHUNK)

            # batch-0 input on SP first; head weights ride DVE so their
            # SEQ/DGE setup overlaps the input's instead of preceding it
            x0 = xspool.tile([128, 2, XC], f16)
            nc.sync.dma_start(out=x0[:, 0:1, :], in_=xh_d[0][:, 0:1, :])
            nc.scalar.dma_start(out=wsb[:, 0:HEAD], in_=wsb_d[:, 0:HEAD])
            nc.sync.dma_start(out=wsb[:, HEAD:], in_=wsb_d[:, HEAD:])

            we_sb_t = consts.tile([CIN, CE_W + CE_X], f16)
            nc.sync.dma_start(out=we_sb_t[:], in_=we_d[:])
            we_sb = we_sb_t[:, 0:CE_W].rearrange(
                "c (s e o) -> c s e o", s=2, e=NDELTA)
            xe_sb = we_sb_t[:, CE_W:CE_W + CE_X].rearrange(
                "c (s e b) -> c s e b", s=2, e=NDELTA)
            nc.sync.dma_start(out=x0[:, 1:2, :], in_=xh_d[0][:, 1:2, :])
            x1 = xspool.tile([128, 2, XC], f16)
            nc.sync.dma_start(out=x1[:, 0:1, :], in_=xh_d[1][:, 0:1, :])
            nc.sync.dma_start(out=x1[:, 1:2, :], in_=xh_d[1][:, 1:2, :])

            def xpair(p):
                xt = xspool.tile([128, 2, XC], f16)
                nc.sync.dma_start(out=xt[:], in_=xh_d[p])
                return xt

            ob = opool.tile([COUT, BL, T], f16)

            def conv(xt, b0, nb, boff, c0=0, nc_=T):
                # boff: batch index of xt[:, b0] within ob; out col window
                # [c0, c0+nc_)
                xv = xt[:].rearrange("p b (u s) -> p b u s", s=KC)
                t1 = ps1pool.tile([COUT, nb, nc_], f32)
                # edge-delta corrections accumulate straight into PSUM cols
                # 0/255 (N=2 matmuls are ~free on the PE), keeping the
                # post-chain path a single ACT copy.
                sides = [s for s, on in ((0, c0 == 0), (1, c0 + nc_ == T))
                         if on]
                nmm = sum(len(g[2]) for g in GROUPS) + NDELTA * len(sides)
                k = 0
                for slot, u0, idxs in GROUPS:
                    for n, i in enumerate(idxs):
                        nc.tensor.matmul(
                            t1[:], lhsT=wv[:, slot + n, :],
                            rhs=xv[:, b0:b0 + nb, u0 + c0:u0 + c0 + nc_, i],
                            start=(k == 0), stop=False)
                        k += 1
                for side in sides:
                    col = 0 if side == 0 else nc_ - 1
                    xe = xe_sb[:, side, :, boff:boff + nb].rearrange(
                        "c e (b x) -> c e b x", x=1)
                    for e in range(NDELTA):
                        k += 1
                        nc.tensor.matmul(
                            t1[:, :, col:col + 1],
                            lhsT=we_sb[:, side, e, :], rhs=xe[:, e],
                            start=False, stop=(k == nmm))

                o = ob[:, boff:boff + nb, c0:c0 + nc_]
                if o_["assembly"] == "split" and nc_ > 128:
                    h = nc_ // 2
                    nc.scalar.copy(o[:, :, 0:h], t1[:, :, 0:h])
                    nc.vector.tensor_scalar_add(o[:, :, h:nc_], t1[:, :, h:nc_],
                                                0.0)
                else:
                    nc.scalar.copy(o, t1[:])

            if o_["head_colhalf"]:  # single-batch chains while DMAs ramp
                conv(x0, 0, 1, 0, 0, 128)
                conv(x0, 0, 1, 0, 128, 128)
            else:
                conv(x0, 0, 1, 0)
            conv(x0, 1, 1, 1)
            conv(x1, 0, 1, 2)
            conv(x1, 1, 1, 3)
            for p in range(2, NPAIR - 1):
                conv(xpair(p), 0, 2, 2 * p)
                if p == 3:
                    nc.sync.dma_start(out=out_d[:, 0:8, :], in_=ob[:, 0:8, :])
            xl = xpair(NPAIR - 1)   # last pair split: shortens the final copy
            conv(xl, 0, 1, BL - 2)
            nc.sync.dma_start(out=out_d[:, 8:15, :], in_=ob[:, 8:15, :])
            tc_ = o_["tail_cols"]
            if tc_ == T:
                conv(xl, 1, 1, BL - 1)
            else:
                conv(xl, 1, 1, BL - 1, 0, T - tc_)
                conv(xl, 1, 1, BL - 1, T - tc_, tc_)
            nc.sync.dma_start(out=out_d[:, 15:BL, :], in_=ob[:, 15:BL, :])

    nc.compile()
    return nc


def _get_state():
    if "nc" not in _STATE:
        _STATE["nc"] = _build_device()
    return _STATE["nc"]


# ---------------------------------------------------------------------------
# entry point
# ---------------------------------------------------------------------------

def _host_pack(C, inp, off):
    """Marshal composed weights + inputs into the device tensors.
    Returns (wsb [128, NCHUNK*COUT], wedge [CIN,2,ND,COUT],
    xedge [B,CIN,2,ND], xh [B/2,128,2,XC])."""
    wint, bint = C["wint"], C["bint"]

    # block weight maps: A (taps 0-6), B (taps 7-14, + bias col), C (15-21)
    Am = np.zeros((COUT, BK))
    Bm = np.zeros((COUT, BK))
    Cm = np.zeros((COUT, BK))
    for m in range(NTAP):
        if m < 7:
            Am[:, 88 * (m + 1):88 * (m + 2)] = wint[m]
        elif m < 15:
            Bm[:, 88 * (m - 7):88 * (m - 6)] = wint[m]
        else:
            Cm[:, 88 * (m - 15):88 * (m - 14)] = wint[m]
    Bm[:, 704] = bint
    assert np.all(Cm[:, 640:] == 0.0)
    # A's chunk-0 rows (block cols 88..128) ride the duplicated pad rows
    Am2 = Am.copy()
    Am2[:, 705:745] = Am[:, 88:128]
    Am2[:, :128] = 0.0

    wsb = np.zeros((128, NCHUNK, COUT), np.float16)
    for (slot, _, idxs), M in zip(GROUPS, (Bm, Am2, Cm)):
        for n, i in enumerate(idxs):
            wsb[:, slot + n, :] = M[:, 128 * i:128 * i + 128].T
    wsb = wsb.reshape(128, NCHUNK * COUT)

    # input marshalling: [B, F, 88] -> padded blocks -> partition-major
    x88 = np.concatenate([inp, off], -1).reshape(B, F, CIN)
    xb = np.zeros((B, UB, BK), np.float16)
    xb[:, 1:257, :704] = x88.reshape(B, T, 704)
    xb[:, 1:257, 704] = 1.0
    xb[:, :, 705:745] = xb[:, :, 88:128]
    xh = np.ascontiguousarray(
        xb.reshape(B // 2, 2, UB, KC, 128).transpose(0, 4, 1, 2, 3)
    ).reshape(B // 2, 128, 2, XC)

    # edge delta weights/inputs
    x88T = x88.transpose(0, 2, 1)                                # [B, 88, F]
    xedge = np.zeros((B, CIN, 2, NDELTA), np.float16)
    xedge[:, :, 0, :3] = x88T[:, :, :3]
    xedge[:, :, 1, :3] = x88T[:, :, F - 3:]
    xedge[:, 0, :, 3] = 1.0

    dwl = (C["wl"][:3] - wint[7:10]).transpose(2, 0, 1)          # [88, 3, 66]
    dwr = (C["wr"][12:15] - wint[12:15]).transpose(2, 0, 1)
    wedge = np.zeros((CIN, 2, NDELTA, COUT), np.float16)
    wedge[:, 0, :3, :] = dwl
    wedge[:, 1, :3, :] = dwr
    wedge[0, 0, 3, :] = C["bl"] - bint
    wedge[0, 1, 3, :] = C["br"] - bint
    return wsb, wedge, xedge, xh


def _core_we(wedge, xedge, c):
    s = slice(c * BL, (c + 1) * BL)
    return np.concatenate([
        wedge.reshape(CIN, -1),
        np.ascontiguousarray(
            xedge[s].transpose(1, 2, 3, 0)).reshape(CIN, -1),
    ], axis=1)


def _kernel_impl(**inputs):
    from concourse.bass_utils import run_bass_kernel_spmd

    P = {k: np.asarray(v) for k, v in inputs.items()}
    inp = P.pop("input").astype(np.float32, copy=False)
    off = P.pop("offset").astype(np.float32, copy=False)

    wsb, wedge, xedge, xh = _host_pack(_compose(P), inp, off)

    in_maps = []
    for c in range(NCORES):
        in_maps.append({
            "wsb": wsb,
            "we": _core_we(wedge, xedge, c),
            "xh": xh[c * NPAIR:(c + 1) * NPAIR],
        })

    nc = _get_state()
    res = run_bass_kernel_spmd(nc, in_maps, core_ids=list(range(NCORES)))

    out = np.empty((B, T, J, POS), np.float32)
    for c in range(NCORES):
        o = res.results[c]["out"].astype(np.float32)             # [66, BL, 256]
        out[c * BL:(c + 1) * BL] = o.transpose(1, 2, 0).reshape(BL, T, J, POS)
    return out


def _subproc_main(in_path, out_path):
    with open(in_path, "rb") as f:
        import pickle
        inputs = pickle.load(f)
    np.save(out_path, _kernel_impl(**inputs))


def kernel(**inputs):
    """Entry point. The very first execution of a freshly compiled NEFF
    occasionally kills the device session (NRT_EXEC_UNIT_UNRECOVERABLE);
    a rerun in a fresh process reliably succeeds (the compile cache makes
    it cheap). So: try in-process, fall back to fresh subprocesses."""
    if not _STATE.get("dead"):
        try:
            return _kernel_impl(**inputs)
        except Exception:  # noqa: BLE001
            _STATE["dead"] = True  # this process's device session is gone

    import pickle
    import subprocess
    import tempfile

    kdir = os.path.dirname(os.path.abspath(__file__))
    last_err = None
    for _ in range(3):
        with tempfile.TemporaryDirectory() as td:
            ip = os.path.join(td, "in.pkl")
            op = os.path.join(td, "out.npy")
            with open(ip, "wb") as f:
                pickle.dump({k: np.asarray(v) for k, v in inputs.items()}, f,
                            protocol=4)
            code = (
                "import sys; sys.path.insert(0, {kd!r}); import kernel; "
                "kernel._subproc_main({ip!r}, {op!r})"
            ).format(kd=kdir, ip=ip, op=op)
            r = subprocess.run([sys.executable, "-c", code],
                               capture_output=True, text=True)
            if r.returncode == 0 and os.path.exists(op):
                return np.load(op)
            last_err = r.stderr[-2000:] if r.stderr else f"rc={r.returncode}"
    raise RuntimeError(f"kernel subprocess retries exhausted: {last_err}")



# revision 4
# speedup vs baseline: 1.2322x; 1.2322x over previous
"""Trainium2 Bass kernel for nn_Encoder_29661044146233 (gnn_message_passing).

Approach
--------
The network is linear per output frame, so it folds into a single 22-tap
stride-8 conv (88 -> 66 channels) whose weights are probed on the host in
float64 (see _compose).  The composed map out[t] = A xblk[t-1] + B xblk[t]
+ C xblk[t+1] runs on 8-frame input blocks (704 values zero-padded to
768 = 6*128 so the contraction tiles the full 128-partition dim).

This version runs the matmuls in fp8e4 with the DoubleRow perf mode
(2 K-tiles of 128 per instruction at 0.5 cycles/row), which the TRN2 ISA
permits for M <= 64 and 16-byte-aligned weight pair strides.  The device
therefore computes output rows 0..63; the remaining 2 of the 66 channels
are computed exactly on the host (cheap numpy) and stitched in.

fp8 e4m3 alone is ~3.8% off, so the contraction is error-corrected:

    W x ~= Whi xhi  +  Wlo xhi  +  Whi xlo

with Whi = e4m3(W), Wlo = e4m3(W - Whi), xhi = e4m3(x), xlo = e4m3(x-xhi)
(all at global power-of-2 scales to dodge the e4m3 subnormal floor).  The
two correction terms are band-limited to the high-energy taps (the exact
chunk pairs already present in the main term), which measures ~0.8%
end-to-end vs the 2e-2 tolerance.  Per batch: 9 DR (main) + 7 DR (W corr)
+ 7 DR (x corr) accumulating into one PSUM tile, plus N=1 fp16 edge-delta
matmuls for the boundary columns.  The x-corr reuses the main term's
weight slots, so the weight blob stays small.

Inputs go to the device as two stacked fp8 block arrays (hi, lo) per
batch; out is fp16 (scaled back by 2^-15 during the PSUM->SBUF copy).
"""

import os
import sys

for _p in ("/opt/trn_rl_repo", "/root/.axon_site/_ro/trn_rl_repo"):
    if os.path.isdir(_p) and _p not in sys.path:
        sys.path.append(_p)

import numpy as np
import ml_dtypes

TOPOLOGY = [0, 0, 1, 2, 3, 4, 0, 6, 7, 8, 0, 10, 11, 12, 12, 14, 15, 16, 12, 18, 19, 20]
J = 22
POS, OFF = 3, 1
CIN = 88
COUT = 66
MD = 64                   # device-computed output rows (DoubleRow M cap)
NTAP = 22
NEDGE = 15
B, F, T = 128, 2048, 256
NCORES = 8
BL = B // NCORES          # batch per core
NPAIR = BL // 2
UB = 258                  # blocks incl one zero pad each side
BK = 768                  # padded block length (704 data + 64 zero pad)
KC = 6                    # K chunks of 128 per block
XC = UB * KC              # sbuf cols per batch per array (hi or lo)
SW = 2.0 ** 10            # weight scale before e4m3
SX = 2.0 ** 5             # input scale before e4m3
SOUT = 1.0 / (SW * SX)

# DoubleRow slot tables: (slot, u0, c) with u0 the rhs block-window (0=A/x[t-1],
# 1=B/x[t], 2=C/x[t+1]) and c the even base chunk of the (c, c+1) pair.
# slot 8 pairs C-chunk4 with zero weights (C has 5 nonzero chunks).
MAIN = [(0, 1, 0), (1, 1, 2), (2, 1, 4),
        (3, 0, 0), (4, 0, 2), (5, 0, 4),
        (6, 2, 0), (7, 2, 2), (8, 2, 4)]
WCORR = [(9, 1, 0), (10, 1, 2), (11, 1, 4),
         (12, 0, 2), (13, 0, 4),
         (14, 2, 0), (15, 2, 2)]
XCORR = [(0, 1, 0), (1, 1, 2), (2, 1, 4),
         (4, 0, 2), (5, 0, 4),
         (6, 2, 0), (7, 2, 2)]
NSLOT = 16
ND = 3                    # edge-delta frames per side
CE_W = 2 * ND * MD
CE_X = 2 * ND * BL


# ---------------------------------------------------------------------------
# host-side weight composition (float64 impulse probing) — unchanged
# ---------------------------------------------------------------------------

def _adj():
    a = np.zeros((J, J), np.float64)
    for i, p in enumerate(TOPOLOGY):
        if i:
            a[p, i] = 1.0
    return a


def _conv_np(z, w, b):
    Bn, Fn, C = z.shape
    zp = np.zeros((Bn, Fn + 2, C), z.dtype)
    zp[:, 1:Fn + 1] = z
    Fo = Fn // 2
    out = np.zeros((Bn, Fo, w.shape[0]), z.dtype)
    for k in range(4):
        out += zp[:, k:k + 2 * Fo:2] @ w[:, :, k].T
    return out + b


def _graph_mat(A, n2n_w, n2n_b, e2n_we, e2n_wn, e2n_b,
               n2e_wn, n2e_we, n2e_b, lin_w, lin_b):
    def apply(z):
        sh = z.shape[:-1]
        zz = z.reshape(-1, J, 4)
        node, edge = zz[..., :POS], zz[..., POS:]
        agg_n = np.einsum('ij,bjc->bic', A, node)
        agg_e = np.einsum('ij,bjc->bic', A, edge)
        f1 = agg_n @ n2n_w + n2n_b
        f2 = agg_e @ e2n_we + node @ e2n_wn + e2n_b
        new_edge = (np.einsum('ji,bjc->bic', A, node) @ n2e_wn
                    + edge @ n2e_we + n2e_b)
        h = np.concatenate([f1, f2], axis=-1) @ lin_w + lin_b
        return np.concatenate([h, new_edge], axis=-1).reshape(*sh, 88)

    g = apply(np.zeros((1, 88)))[0]
    G = apply(np.eye(88)) - g
    return G.T, g


def _compose(P):
    A = _adj()
    P64 = {k: np.asarray(v, np.float64) for k, v in P.items()}
    gnames = ('n2n_w', 'n2n_b', 'e2n_we', 'e2n_wn', 'e2n_b',
              'n2e_wn', 'n2e_we', 'n2e_b', 'lin_w', 'lin_b')
    G1, g1 = _graph_mat(A, *[P64['g1_' + s] for s in gnames])
    G2, g2 = _graph_mat(A, *[P64['g2_' + s] for s in gnames])
    keep = np.array([4 * j + c for j in range(J) for c in range(POS)])

    def pipeline(x88):
        y = _conv_np(x88, P64['conv1_w'], P64['conv1_b'])
        y = y @ G1.T + g1
        y = _conv_np(y, P64['conv2_w'], P64['conv2_b'])
        y = y @ G2.T + g2
        y = _conv_np(y, P64['conv3_w'], P64['conv3_b'])
        return y[..., keep]

    Fp = 256
    Tp = Fp // 8
    zb = pipeline(np.zeros((1, Fp, 88)))[0]
    bint, bl, br = zb[Tp // 2], zb[0], zb[Tp - 1]

    mid = Fp // 2
    probes = np.zeros((8 * 88, Fp, 88))
    for r in range(8):
        for ic in range(88):
            probes[r * 88 + ic, mid + r, ic] = 1.0
    resp = pipeline(probes) - zb
    wint = np.zeros((NTAP, COUT, CIN))
    for r in range(8):
        for t in range(Tp):
            m = (mid + r) - 8 * t + 7
            if 0 <= m < NTAP:
                wint[m] = resp[r * 88:(r + 1) * 88, t, :].T

    probes = np.zeros((NEDGE * 88, Fp, 88))
    for f in range(NEDGE):
        for ic in range(88):
            probes[f * 88 + ic, f, ic] = 1.0
    resp = pipeline(probes) - zb
    wl = np.stack([resp[f * 88:(f + 1) * 88, 0, :].T for f in range(NEDGE)])

    probes = np.zeros((NEDGE * 88, Fp, 88))
    for f in range(NEDGE):
        for ic in range(88):
            probes[f * 88 + ic, Fp - NEDGE + f, ic] = 1.0
    resp = pipeline(probes) - zb
    wr = np.stack([resp[f * 88:(f + 1) * 88, Tp - 1, :].T for f in range(NEDGE)])

    return dict(wint=wint, bint=bint, wl=wl, wr=wr, bl=bl, br=br)


# ---------------------------------------------------------------------------
# device program (built/compiled once, reused across calls)
# ---------------------------------------------------------------------------

_STATE = {}

DEFAULT_OPTS = dict(
    warm_n=12,          # warm-up matmul count (bridge p-state ramp)
    memset_cols=2,
    xs_bufs=5,
    tail_cols=64,       # final col-chain width (T = no split)
)


def _build_device(opts=None):
    import concourse.bass as bass  # noqa: F401
    import concourse.tile as tile
    from concourse import bacc, mybir

    o_ = dict(DEFAULT_OPTS)
    if opts:
        o_.update(opts)
    f32 = mybir.dt.float32
    f16 = mybir.dt.float16
    f8 = mybir.dt.float8e4
    DR = mybir.MatmulPerfMode.DoubleRow
    nc = bacc.Bacc("TRN2", target_bir_lowering=False, debug=False,
                   num_devices=NCORES)

    wsb_d = nc.dram_tensor("wsb", [128, NSLOT * 128], f8, kind="ExternalInput")
    we_d = nc.dram_tensor("we", [CIN, CE_W + CE_X], f16, kind="ExternalInput")
    xh_d = nc.dram_tensor("xh", [NPAIR, 128, 2, 2 * XC], f8, kind="ExternalInput")
    out_d = nc.dram_tensor("out", [MD, BL, T], f16, kind="ExternalOutput")

    with tile.TileContext(nc) as tc:
        with (
            tc.tile_pool(name="consts", bufs=1) as consts,
            tc.tile_pool(name="xs", bufs=o_["xs_bufs"]) as xspool,
            tc.tile_pool(name="ps1", bufs=4, space="PSUM") as ps1pool,
            tc.tile_pool(name="warm", bufs=1, space="PSUM") as warmpool,
            tc.tile_pool(name="ob", bufs=1) as opool,
        ):
            # PE warm-up: dummy bf16 matmuls on scratch, no DMA deps, to
            # bridge the ~3us p-state ramp while the first DMAs stream.
            bf16 = mybir.dt.bfloat16
            scratch = consts.tile([CIN, 162], f32)
            if o_["memset_cols"]:
                nc.vector.memset(scratch[:, 0:o_["memset_cols"]], 0.0)
            s16 = scratch[:].bitcast(bf16)
            wps = warmpool.tile([COUT, 256], f32)
            for _ in range(o_["warm_n"]):
                nc.tensor.matmul(wps[:], lhsT=s16[:, 0:COUT],
                                 rhs=s16[:, 66:322], start=True, stop=True)

            # DMA order: weight blob + batch-0 input first, then edge blob,
            # then per-pair input streams.
            wsb = consts.tile([128, NSLOT, 2, MD], f8)
            x0 = xspool.tile([128, 2, 2, XC], f8)
            nc.sync.dma_start(out=x0[:, 0], in_=xh_d[0][:, 0].rearrange(
                "p (a x) -> p a x", a=2))
            nc.scalar.dma_start(out=wsb[:], in_=wsb_d[:])
            we_sb_t = consts.tile([CIN, CE_W + CE_X], f16)
            nc.scalar.dma_start(out=we_sb_t[:], in_=we_d[:])
            nc.sync.dma_start(out=x0[:, 1], in_=xh_d[0][:, 1].rearrange(
                "p (a x) -> p a x", a=2))
            we_sb = we_sb_t[:, 0:CE_W].rearrange(
                "c (s e o) -> c s e o", s=2, e=ND)
            xe_sb = we_sb_t[:, CE_W:CE_W + CE_X].rearrange(
                "c (s e b) -> c s e b", s=2, e=ND)

            def xpair(p):
                xt = xspool.tile([128, 2, 2, XC], f8)
                nc.sync.dma_start(
                    out=xt[:],
                    in_=xh_d[p].rearrange("p b (a x) -> p b a x", a=2))
                return xt

            ob = opool.tile([MD, BL, T], f16)

            def conv(xt, b0, boff, c0=0, nc_=T):
                # one batch; out col window [c0, c0+nc_)
                xv = xt[:].rearrange("p b a (u s) -> p b a u s", s=KC)
                t1 = ps1pool.tile([MD, nc_], f32)
                sides = [s for s, on in ((0, c0 == 0), (1, c0 + nc_ == T))
                         if on]
                nmm = len(MAIN) + len(WCORR) + len(XCORR) + ND * len(sides)
                k = 0

                def rhs(a, u0, c):
                    return xv[:, b0, a, u0 + c0:u0 + c0 + nc_, c:c + 2] \
                        .rearrange("p u s -> p s u")

                for table, a in ((MAIN, 0), (WCORR, 0), (XCORR, 1)):
                    for slot, u0, c in table:
                        nc.tensor.matmul(
                            t1[:], lhsT=wsb[:, slot], rhs=rhs(a, u0, c),
                            start=(k == 0), stop=False, perf_mode=DR)
                        k += 1
                for side in sides:
                    col = 0 if side == 0 else nc_ - 1
                    xe = xe_sb[:, side, :, boff:boff + 1].rearrange(
                        "c e (b x) -> c e b x", x=1)
                    for e in range(ND):
                        k += 1
                        nc.tensor.matmul(
                            t1[:, col:col + 1],
                            lhsT=we_sb[:, side, e, :], rhs=xe[:, e],
                            start=False, stop=(k == nmm))

                o = ob[:, boff:boff + 1, c0:c0 + nc_].rearrange("m b n -> m (b n)")
                nc.scalar.activation(
                    o, t1[:], mybir.ActivationFunctionType.Identity,
                    bias=0.0, scale=SOUT)

            conv(x0, 0, 0)
            conv(x0, 1, 1)
            for p in range(1, NPAIR - 1):
                xt = xpair(p)
                conv(xt, 0, 2 * p)
                if p == 3:
                    nc.sync.dma_start(out=out_d[:, 0:7, :], in_=ob[:, 0:7, :])
                conv(xt, 1, 2 * p + 1)
            xl = xpair(NPAIR - 1)   # last pair split: shortens the final copy
            conv(xl, 0, BL - 2)
            nc.sync.dma_start(out=out_d[:, 7:15, :], in_=ob[:, 7:15, :])
            tc_ = o_["tail_cols"]
            if tc_ == T:
                conv(xl, 1, BL - 1)
            else:
                conv(xl, 1, BL - 1, 0, T - tc_)
                conv(xl, 1, BL - 1, T - tc_, tc_)
            nc.sync.dma_start(out=out_d[:, 15:BL, :], in_=ob[:, 15:BL, :])

    nc.compile()
    return nc


def _get_state():
    if "nc" not in _STATE:
        _STATE["nc"] = _build_device()
    return _STATE["nc"]


# ---------------------------------------------------------------------------
# host packing
# ---------------------------------------------------------------------------

def _fp8(v):
    return np.asarray(v, dtype=ml_dtypes.float8_e4m3fn)


def _host_pack(C, x88):
    """Marshal composed weights + inputs into the device tensors."""
    wint = C["wint"]

    Am = np.zeros((COUT, BK))
    Bm = np.zeros((COUT, BK))
    Cm = np.zeros((COUT, BK))
    for m in range(NTAP):
        if m < 7:
            Am[:, 88 * (m + 1):88 * (m + 2)] = wint[m]
        elif m < 15:
            Bm[:, 88 * (m - 7):88 * (m - 6)] = wint[m]
        else:
            Cm[:, 88 * (m - 15):88 * (m - 14)] = wint[m]
    maps = {0: Am, 1: Bm, 2: Cm}
    hi = {}
    lo = {}
    for u0, M in maps.items():
        h = _fp8(M * SW)
        hi[u0] = h
        lo[u0] = _fp8(M * SW - h.astype(np.float64))

    wsb = np.zeros((128, NSLOT, 2, MD), ml_dtypes.float8_e4m3fn)
    for slot, u0, c in MAIN:
        for j in range(2):
            cc = c + j
            if cc < KC and not (u0 == 2 and cc == 5):
                wsb[:, slot, j, :] = hi[u0][:MD, 128 * cc:128 * cc + 128].T
    for slot, u0, c in WCORR:
        for j in range(2):
            cc = c + j
            if cc < KC and not (u0 == 2 and cc == 5):
                wsb[:, slot, j, :] = lo[u0][:MD, 128 * cc:128 * cc + 128].T
    wsb = wsb.reshape(128, NSLOT * 128)

    # input marshalling: [B, F, 88] -> scaled hi/lo padded blocks
    xb = np.zeros((B, UB, BK))
    xb[:, 1:257, :704] = x88.reshape(B, T, 704) * SX
    xhi = _fp8(xb)
    xlo = _fp8(xb - xhi.astype(np.float64))
    xs = np.stack([xhi, xlo], axis=1)        # [B, 2, UB, BK]
    xh = np.ascontiguousarray(
        xs.reshape(B // 2, 2, 2, UB, KC, 128).transpose(0, 5, 1, 2, 3, 4)
    ).reshape(B // 2, 128, 2, 2 * XC)

    # edge delta weights/inputs (fp16, scaled to match the fp8 terms)
    x88T = x88.transpose(0, 2, 1)                                # [B, 88, F]
    xedge = np.zeros((B, CIN, 2, ND), np.float16)
    xedge[:, :, 0, :] = x88T[:, :, :ND] * SX
    xedge[:, :, 1, :] = x88T[:, :, F - ND:] * SX

    dwl = (C["wl"][:3] - wint[7:10]).transpose(2, 0, 1)          # [88, 3, 66]
    dwr = (C["wr"][12:15] - wint[12:15]).transpose(2, 0, 1)
    wedge = np.zeros((CIN, 2, ND, MD), np.float16)
    wedge[:, 0, :, :] = dwl[:, :, :MD] * SW
    wedge[:, 1, :, :] = dwr[:, :, :MD] * SW
    return wsb, wedge, xedge, xh


def _core_we(wedge, xedge, c):
    s = slice(c * BL, (c + 1) * BL)
    return np.concatenate([
        wedge.reshape(CIN, -1),
        np.ascontiguousarray(
            xedge[s].transpose(1, 2, 3, 0)).reshape(CIN, -1),
    ], axis=1)


def _host_tail(C, x88):
    """Exact host computation of output channels MD..66 plus the bias
    terms (all-zero for the given inputs, kept for generality)."""
    wint, wl, wr = C["wint"], C["wl"], C["wr"]
    xp = np.zeros((B, F + 16, CIN))
    xp[:, 7:7 + F] = x88
    h2 = np.zeros((B, T, COUT - MD))
    for m in range(NTAP):
        h2 += xp[:, m:m + 8 * T:8] @ wint[m, MD:COUT].T
    dwl = wl[:3] - wint[7:10]
    dwr = wr[12:15] - wint[12:15]
    for e in range(ND):
        h2[:, 0] += x88[:, e] @ dwl[e, MD:COUT].T
        h2[:, T - 1] += x88[:, F - ND + e] @ dwr[e, MD:COUT].T
    h2 += C["bint"][MD:COUT]
    h2[:, 0] += (C["bl"] - C["bint"])[MD:COUT]
    h2[:, T - 1] += (C["br"] - C["bint"])[MD:COUT]
    return h2


# ---------------------------------------------------------------------------
# entry point
# ---------------------------------------------------------------------------

def _kernel_impl(**inputs):
    from concourse.bass_utils import run_bass_kernel_spmd

    P = {k: np.asarray(v) for k, v in inputs.items()}
    inp = P.pop("input").astype(np.float64, copy=False)
    off = P.pop("offset").astype(np.float64, copy=False)
    x88 = np.concatenate([inp, off], -1).reshape(B, F, CIN)

    C = _compose(P)
    wsb, wedge, xedge, xh = _host_pack(C, x88)
    h2 = _host_tail(C, x88)

    in_maps = []
    for c in range(NCORES):
        in_maps.append({
            "wsb": wsb,
            "we": _core_we(wedge, xedge, c),
            "xh": xh[c * NPAIR:(c + 1) * NPAIR],
        })

    nc = _get_state()
    res = run_bass_kernel_spmd(nc, in_maps, core_ids=list(range(NCORES)))

    bias = C["bint"][:MD]
    bl = (C["bl"] - C["bint"])[:MD]
    br = (C["br"] - C["bint"])[:MD]
    out = np.empty((B, T, COUT), np.float32)
    for c in range(NCORES):
        o = res.results[c]["out"].astype(np.float32)             # [64, BL, 256]
        out[c * BL:(c + 1) * BL, :, :MD] = o.transpose(1, 2, 0)
    out[:, :, :MD] += bias
    out[:, 0, :MD] += bl
    out[:, T - 1, :MD] += br
    out[:, :, MD:] = h2
    return out.reshape(B, T, J, POS)


def _subproc_main(in_path, out_path):
    with open(in_path, "rb") as f:
        import pickle
        inputs = pickle.load(f)
    np.save(out_path, _kernel_impl(**inputs))


def kernel(**inputs):
    """Entry point. The very first execution of a freshly compiled NEFF
    occasionally kills the device session (NRT_EXEC_UNIT_UNRECOVERABLE);
    a rerun in a fresh process reliably succeeds (the compile cache makes
    it cheap). So: try in-process, fall back to fresh subprocesses."""
    if not _STATE.get("dead"):
        try:
            return _kernel_impl(**inputs)
        except Exception:  # noqa: BLE001
            _STATE["dead"] = True  # this process's device session is gone

    import pickle
    import subprocess
    import tempfile

    kdir = os.path.dirname(os.path.abspath(__file__))
    last_err = None
    for _ in range(3):
        with tempfile.TemporaryDirectory() as td:
            ip = os.path.join(td, "in.pkl")
            op = os.path.join(td, "out.npy")
            with open(ip, "wb") as f:
                pickle.dump({k: np.asarray(v) for k, v in inputs.items()}, f,
                            protocol=4)
            code = (
                "import sys; sys.path.insert(0, {kd!r}); import kernel; "
                "kernel._subproc_main({ip!r}, {op!r})"
            ).format(kd=kdir, ip=ip, op=op)
            r = subprocess.run([sys.executable, "-c", code],
                               capture_output=True, text=True)
            if r.returncode == 0 and os.path.exists(op):
                return np.load(op)
            last_err = r.stderr[-2000:] if r.stderr else f"rc={r.returncode}"
    raise RuntimeError(f"kernel subprocess retries exhausted: {last_err}")


# revision 11
# speedup vs baseline: 1.2782x; 1.0373x over previous
"""Trainium2 Bass kernel for nn_Encoder_29661044146233 (gnn_message_passing).

Approach
--------
The network is linear per output frame, so it folds into a single 22-tap
stride-8 conv (88 -> 66 channels) whose weights are probed on the host in
float64 (see _compose).  The composed map out[t] = A xblk[t-1] + B xblk[t]
+ C xblk[t+1] runs on 8-frame input blocks (704 values zero-padded to
768 = 6*128 so the contraction tiles the full 128-partition dim).

This version runs the matmuls in fp8e4 with the DoubleRow perf mode
(2 K-tiles of 128 per instruction at 0.5 cycles/row), which the TRN2 ISA
permits for M <= 64 and 16-byte-aligned weight pair strides.  The device
therefore computes output rows 0..63; the remaining 2 of the 66 channels
are computed exactly on the host (cheap numpy) and stitched in.

fp8 e4m3 alone is ~3.8% off, so the contraction is error-corrected:

    W x ~= Whi xhi  +  Wlo xhi  +  Whi xlo

with Whi = e4m3(W), Wlo = e4m3(W - Whi), xhi = e4m3(x), xlo = e4m3(x-xhi)
(all at global power-of-2 scales to dodge the e4m3 subnormal floor).  The
two correction terms are band-limited to the high-energy taps (the exact
chunk pairs already present in the main term), which measures ~0.8%
end-to-end vs the 2e-2 tolerance.  Per batch: 9 DR (main) + 7 DR (W corr)
+ 7 DR (x corr) accumulating into one PSUM tile, plus N=1 fp16 edge-delta
matmuls for the boundary columns.  The x-corr reuses the main term's
weight slots, so the weight blob stays small.

Inputs go to the device as two stacked fp8 block arrays (hi, lo) per
batch; out is fp16 (scaled back by 2^-15 during the PSUM->SBUF copy).
"""

import os
import sys

for _p in ("/opt/trn_rl_repo", "/root/.axon_site/_ro/trn_rl_repo"):
    if os.path.isdir(_p) and _p not in sys.path:
        sys.path.append(_p)

import numpy as np
import ml_dtypes

TOPOLOGY = [0, 0, 1, 2, 3, 4, 0, 6, 7, 8, 0, 10, 11, 12, 12, 14, 15, 16, 12, 18, 19, 20]
J = 22
POS, OFF = 3, 1
CIN = 88
COUT = 66
MD = 64                   # device-computed output rows (DoubleRow M cap)
NTAP = 22
NEDGE = 15
B, F, T = 128, 2048, 256
NCORES = 8
BL = B // NCORES          # batch per core
NPAIR = BL // 2
UB = 258                  # blocks incl one zero pad each side
BK = 768                  # padded block length (704 data + 64 zero pad)
KC = 6                    # K chunks of 128 per block
XC = UB * KC              # sbuf cols per batch per array (hi or lo)
SW = 2.0 ** 10            # weight scale before e4m3
SX = 2.0 ** 5             # input scale before e4m3
SOUT = 1.0 / (SW * SX)

# DoubleRow slot tables: (slot, u0, c) with u0 the rhs block-window (0=A/x[t-1],
# 1=B/x[t], 2=C/x[t+1]) and c the even base chunk of the (c, c+1) pair.
# slot 8 pairs C-chunk4 with zero weights (C has 5 nonzero chunks).
MAIN = [(0, 1, 0), (1, 1, 2), (2, 1, 4),
        (3, 0, 0), (4, 0, 2), (5, 0, 4),
        (6, 2, 0), (7, 2, 2), (8, 2, 4)]
WCORR = [(9, 1, 0), (10, 1, 2), (11, 1, 4),
         (12, 0, 2), (13, 0, 4),
         (14, 2, 0)]
XCORR = [(0, 1, 0), (1, 1, 2), (2, 1, 4),
         (4, 0, 2), (5, 0, 4),
         (6, 2, 0)]
NSLOT = 15
ND = 3                    # edge-delta frames per side
CE_W = 2 * ND * MD
CE_X = 2 * ND * BL


# ---------------------------------------------------------------------------
# host-side weight composition (float64 impulse probing) — unchanged
# ---------------------------------------------------------------------------

def _adj():
    a = np.zeros((J, J), np.float64)
    for i, p in enumerate(TOPOLOGY):
        if i:
            a[p, i] = 1.0
    return a


def _conv_np(z, w, b):
    Bn, Fn, C = z.shape
    zp = np.zeros((Bn, Fn + 2, C), z.dtype)
    zp[:, 1:Fn + 1] = z
    Fo = Fn // 2
    out = np.zeros((Bn, Fo, w.shape[0]), z.dtype)
    for k in range(4):
        out += zp[:, k:k + 2 * Fo:2] @ w[:, :, k].T
    return out + b


def _graph_mat(A, n2n_w, n2n_b, e2n_we, e2n_wn, e2n_b,
               n2e_wn, n2e_we, n2e_b, lin_w, lin_b):
    def apply(z):
        sh = z.shape[:-1]
        zz = z.reshape(-1, J, 4)
        node, edge = zz[..., :POS], zz[..., POS:]
        agg_n = np.einsum('ij,bjc->bic', A, node)
        agg_e = np.einsum('ij,bjc->bic', A, edge)
        f1 = agg_n @ n2n_w + n2n_b
        f2 = agg_e @ e2n_we + node @ e2n_wn + e2n_b
        new_edge = (np.einsum('ji,bjc->bic', A, node) @ n2e_wn
                    + edge @ n2e_we + n2e_b)
        h = np.concatenate([f1, f2], axis=-1) @ lin_w + lin_b
        return np.concatenate([h, new_edge], axis=-1).reshape(*sh, 88)

    g = apply(np.zeros((1, 88)))[0]
    G = apply(np.eye(88)) - g
    return G.T, g


def _compose(P):
    A = _adj()
    P64 = {k: np.asarray(v, np.float64) for k, v in P.items()}
    gnames = ('n2n_w', 'n2n_b', 'e2n_we', 'e2n_wn', 'e2n_b',
              'n2e_wn', 'n2e_we', 'n2e_b', 'lin_w', 'lin_b')
    G1, g1 = _graph_mat(A, *[P64['g1_' + s] for s in gnames])
    G2, g2 = _graph_mat(A, *[P64['g2_' + s] for s in gnames])
    keep = np.array([4 * j + c for j in range(J) for c in range(POS)])

    def pipeline(x88):
        y = _conv_np(x88, P64['conv1_w'], P64['conv1_b'])
        y = y @ G1.T + g1
        y = _conv_np(y, P64['conv2_w'], P64['conv2_b'])
        y = y @ G2.T + g2
        y = _conv_np(y, P64['conv3_w'], P64['conv3_b'])
        return y[..., keep]

    Fp = 256
    Tp = Fp // 8
    zb = pipeline(np.zeros((1, Fp, 88)))[0]
    bint, bl, br = zb[Tp // 2], zb[0], zb[Tp - 1]

    mid = Fp // 2
    probes = np.zeros((8 * 88, Fp, 88))
    for r in range(8):
        for ic in range(88):
            probes[r * 88 + ic, mid + r, ic] = 1.0
    resp = pipeline(probes) - zb
    wint = np.zeros((NTAP, COUT, CIN))
    for r in range(8):
        for t in range(Tp):
            m = (mid + r) - 8 * t + 7
            if 0 <= m < NTAP:
                wint[m] = resp[r * 88:(r + 1) * 88, t, :].T

    probes = np.zeros((NEDGE * 88, Fp, 88))
    for f in range(NEDGE):
        for ic in range(88):
            probes[f * 88 + ic, f, ic] = 1.0
    resp = pipeline(probes) - zb
    wl = np.stack([resp[f * 88:(f + 1) * 88, 0, :].T for f in range(NEDGE)])

    probes = np.zeros((NEDGE * 88, Fp, 88))
    for f in range(NEDGE):
        for ic in range(88):
            probes[f * 88 + ic, Fp - NEDGE + f, ic] = 1.0
    resp = pipeline(probes) - zb
    wr = np.stack([resp[f * 88:(f + 1) * 88, Tp - 1, :].T for f in range(NEDGE)])

    return dict(wint=wint, bint=bint, wl=wl, wr=wr, bl=bl, br=br)


# ---------------------------------------------------------------------------
# device program (built/compiled once, reused across calls)
# ---------------------------------------------------------------------------

_STATE = {}

DEFAULT_OPTS = dict(
    warm_n=18,          # warm-up matmul count (bridge p-state ramp)
    memset_cols=2,
    xs_bufs=5,
    tail_cols=64,       # final col-chain width (T = no split)
)


def _build_device(opts=None):
    import concourse.bass as bass  # noqa: F401
    import concourse.tile as tile
    from concourse import bacc, mybir

    o_ = dict(DEFAULT_OPTS)
    if opts:
        o_.update(opts)
    f32 = mybir.dt.float32
    f16 = mybir.dt.float16
    f8 = mybir.dt.float8e4
    DR = mybir.MatmulPerfMode.DoubleRow
    nc = bacc.Bacc("TRN2", target_bir_lowering=False, debug=False,
                   num_devices=NCORES)

    wsb_d = nc.dram_tensor("wsb", [128, NSLOT * 128], f8, kind="ExternalInput")
    we_d = nc.dram_tensor("we", [CIN, CE_W + CE_X], f16, kind="ExternalInput")
    xh_d = nc.dram_tensor("xh", [NPAIR, 128, 2, 2 * XC], f8, kind="ExternalInput")
    out_d = nc.dram_tensor("out", [MD, BL, T], f16, kind="ExternalOutput")

    with tile.TileContext(nc) as tc:
        with (
            tc.tile_pool(name="consts", bufs=1) as consts,
            tc.tile_pool(name="xs", bufs=o_["xs_bufs"]) as xspool,
            tc.tile_pool(name="ps1", bufs=4, space="PSUM") as ps1pool,
            tc.tile_pool(name="warm", bufs=1, space="PSUM") as warmpool,
            tc.tile_pool(name="ob", bufs=1) as opool,
        ):
            # PE warm-up: dummy bf16 matmuls on scratch, no DMA deps, to
            # bridge the ~3us p-state ramp while the first DMAs stream.
            bf16 = mybir.dt.bfloat16
            scratch = consts.tile([CIN, 162], f32)
            if o_["memset_cols"]:
                nc.vector.memset(scratch[:, 0:o_["memset_cols"]], 0.0)
            s16 = scratch[:].bitcast(bf16)
            wps = warmpool.tile([COUT, 256], f32)
            for _ in range(o_["warm_n"]):
                nc.tensor.matmul(wps[:], lhsT=s16[:, 0:COUT],
                                 rhs=s16[:, 66:322], start=True, stop=True)

            # DMA order: weight blob + batch-0 input first, then edge blob,
            # then per-pair input streams.
            wsb = consts.tile([128, NSLOT, 2, MD], f8)
            x0 = xspool.tile([128, 2, 2, XC], f8)
            nc.scalar.dma_start(out=wsb[:], in_=wsb_d[:])
            nc.sync.dma_start(out=x0[:, 0], in_=xh_d[0][:, 0].rearrange(
                "p (a x) -> p a x", a=2))
            we_sb_t = consts.tile([CIN, CE_W + CE_X], f16)
            nc.scalar.dma_start(out=we_sb_t[:], in_=we_d[:])
            nc.sync.dma_start(out=x0[:, 1], in_=xh_d[0][:, 1].rearrange(
                "p (a x) -> p a x", a=2))
            we_sb = we_sb_t[:, 0:CE_W].rearrange(
                "c (s e o) -> c s e o", s=2, e=ND)
            xe_sb = we_sb_t[:, CE_W:CE_W + CE_X].rearrange(
                "c (s e b) -> c s e b", s=2, e=ND)

            def xpair(p):
                xt = xspool.tile([128, 2, 2, XC], f8)
                nc.sync.dma_start(
                    out=xt[:],
                    in_=xh_d[p].rearrange("p b (a x) -> p b a x", a=2))
                return xt

            # split output staging into 3 tiles so each store DMA depends
            # only on its own batches (coarse tile deps otherwise park the
            # mid store behind the final batch)
            obA = opool.tile([MD, 7, T], f16)
            obB = opool.tile([MD, 8, T], f16)
            obC = opool.tile([MD, 1, T], f16)

            def obsel(boff):
                if boff < 7:
                    return obA, boff
                if boff < 15:
                    return obB, boff - 7
                return obC, 0

            def conv(xt, b0, boff, c0=0, nc_=T):
                # one batch; out col window [c0, c0+nc_)
                xv = xt[:].rearrange("p b a (u s) -> p b a u s", s=KC)
                t1 = ps1pool.tile([MD, nc_], f32)
                sides = [s for s, on in ((0, c0 == 0), (1, c0 + nc_ == T))
                         if on]
                nmm = len(MAIN) + len(WCORR) + len(XCORR) + ND * len(sides)
                k = 0

                def rhs(a, u0, c):
                    return xv[:, b0, a, u0 + c0:u0 + c0 + nc_, c:c + 2] \
                        .rearrange("p u s -> p s u")

                for table, a in ((MAIN, 0), (WCORR, 0), (XCORR, 1)):
                    for slot, u0, c in table:
                        nc.tensor.matmul(
                            t1[:], lhsT=wsb[:, slot], rhs=rhs(a, u0, c),
                            start=(k == 0), stop=False, perf_mode=DR)
                        k += 1
                for side in sides:
                    col = 0 if side == 0 else nc_ - 1
                    xe = xe_sb[:, side, :, boff:boff + 1].rearrange(
                        "c e (b x) -> c e b x", x=1)
                    for e in range(ND):
                        k += 1
                        nc.tensor.matmul(
                            t1[:, col:col + 1],
                            lhsT=we_sb[:, side, e, :], rhs=xe[:, e],
                            start=False, stop=(k == nmm))

                ot, oi = obsel(boff)
                o = ot[:, oi:oi + 1, c0:c0 + nc_].rearrange("m b n -> m (b n)")
                nc.scalar.activation(
                    o, t1[:], mybir.ActivationFunctionType.Identity,
                    bias=0.0, scale=SOUT)

            conv(x0, 0, 0)
            conv(x0, 1, 1)
            for p in range(1, NPAIR - 1):
                xt = xpair(p)
                conv(xt, 0, 2 * p)
                if p == 3:
                    nc.scalar.dma_start(out=out_d[:, 0:7, :], in_=obA[:])
                conv(xt, 1, 2 * p + 1)
            xl = xpair(NPAIR - 1)   # last pair split: shortens the final copy
            conv(xl, 0, BL - 2)
            nc.scalar.dma_start(out=out_d[:, 7:15, :], in_=obB[:])
            tc_ = o_["tail_cols"]
            if tc_ == T:
                conv(xl, 1, BL - 1)
            else:
                conv(xl, 1, BL - 1, 0, T - tc_)
                conv(xl, 1, BL - 1, T - tc_, tc_)
            nc.sync.dma_start(out=out_d[:, 15:BL, :], in_=obC[:])

    nc.compile()
    return nc


def _get_state():
    if "nc" not in _STATE:
        _STATE["nc"] = _build_device()
    return _STATE["nc"]


# ---------------------------------------------------------------------------
# host packing
# ---------------------------------------------------------------------------

def _fp8(v):
    return np.asarray(v, dtype=ml_dtypes.float8_e4m3fn)


def _host_pack(C, x88):
    """Marshal composed weights + inputs into the device tensors."""
    wint = C["wint"]

    Am = np.zeros((COUT, BK))
    Bm = np.zeros((COUT, BK))
    Cm = np.zeros((COUT, BK))
    for m in range(NTAP):
        if m < 7:
            Am[:, 88 * (m + 1):88 * (m + 2)] = wint[m]
        elif m < 15:
            Bm[:, 88 * (m - 7):88 * (m - 6)] = wint[m]
        else:
            Cm[:, 88 * (m - 15):88 * (m - 14)] = wint[m]
    maps = {0: Am, 1: Bm, 2: Cm}
    hi = {}
    lo = {}
    for u0, M in maps.items():
        h = _fp8(M * SW)
        hi[u0] = h
        lo[u0] = _fp8(M * SW - h.astype(np.float64))

    wsb = np.zeros((128, NSLOT, 2, MD), ml_dtypes.float8_e4m3fn)
    for slot, u0, c in MAIN:
        for j in range(2):
            cc = c + j
            if cc < KC and not (u0 == 2 and cc == 5):
                wsb[:, slot, j, :] = hi[u0][:MD, 128 * cc:128 * cc + 128].T
    for slot, u0, c in WCORR:
        for j in range(2):
            cc = c + j
            if cc < KC and not (u0 == 2 and cc == 5):
                wsb[:, slot, j, :] = lo[u0][:MD, 128 * cc:128 * cc + 128].T
    wsb = wsb.reshape(128, NSLOT * 128)

    # input marshalling: [B, F, 88] -> scaled hi/lo padded blocks
    xb = np.zeros((B, UB, BK))
    xb[:, 1:257, :704] = x88.reshape(B, T, 704) * SX
    xhi = _fp8(xb)
    xlo = _fp8(xb - xhi.astype(np.float64))
    xs = np.stack([xhi, xlo], axis=1)        # [B, 2, UB, BK]
    xh = np.ascontiguousarray(
        xs.reshape(B // 2, 2, 2, UB, KC, 128).transpose(0, 5, 1, 2, 3, 4)
    ).reshape(B // 2, 128, 2, 2 * XC)

    # edge delta weights/inputs (fp16, scaled to match the fp8 terms)
    x88T = x88.transpose(0, 2, 1)                                # [B, 88, F]
    xedge = np.zeros((B, CIN, 2, ND), np.float16)
    xedge[:, :, 0, :] = x88T[:, :, :ND] * SX
    xedge[:, :, 1, :] = x88T[:, :, F - ND:] * SX

    dwl = (C["wl"][:3] - wint[7:10]).transpose(2, 0, 1)          # [88, 3, 66]
    dwr = (C["wr"][12:15] - wint[12:15]).transpose(2, 0, 1)
    wedge = np.zeros((CIN, 2, ND, MD), np.float16)
    wedge[:, 0, :, :] = dwl[:, :, :MD] * SW
    wedge[:, 1, :, :] = dwr[:, :, :MD] * SW
    return wsb, wedge, xedge, xh


def _core_we(wedge, xedge, c):
    s = slice(c * BL, (c + 1) * BL)
    return np.concatenate([
        wedge.reshape(CIN, -1),
        np.ascontiguousarray(
            xedge[s].transpose(1, 2, 3, 0)).reshape(CIN, -1),
    ], axis=1)


def _host_tail(C, x88):
    """Exact host computation of output channels MD..66 plus the bias
    terms (all-zero for the given inputs, kept for generality)."""
    wint, wl, wr = C["wint"], C["wl"], C["wr"]
    xp = np.zeros((B, F + 16, CIN))
    xp[:, 7:7 + F] = x88
    h2 = np.zeros((B, T, COUT - MD))
    for m in range(NTAP):
        h2 += xp[:, m:m + 8 * T:8] @ wint[m, MD:COUT].T
    dwl = wl[:3] - wint[7:10]
    dwr = wr[12:15] - wint[12:15]
    for e in range(ND):
        h2[:, 0] += x88[:, e] @ dwl[e, MD:COUT].T
        h2[:, T - 1] += x88[:, F - ND + e] @ dwr[e, MD:COUT].T
    h2 += C["bint"][MD:COUT]
    h2[:, 0] += (C["bl"] - C["bint"])[MD:COUT]
    h2[:, T - 1] += (C["br"] - C["bint"])[MD:COUT]
    return h2


# ---------------------------------------------------------------------------
# entry point
# ---------------------------------------------------------------------------

def _kernel_impl(**inputs):
    from concourse.bass_utils import run_bass_kernel_spmd

    P = {k: np.asarray(v) for k, v in inputs.items()}
    inp = P.pop("input").astype(np.float64, copy=False)
    off = P.pop("offset").astype(np.float64, copy=False)
    x88 = np.concatenate([inp, off], -1).reshape(B, F, CIN)

    C = _compose(P)
    wsb, wedge, xedge, xh = _host_pack(C, x88)
    h2 = _host_tail(C, x88)

    in_maps = []
    for c in range(NCORES):
        in_maps.append({
            "wsb": wsb,
            "we": _core_we(wedge, xedge, c),
            "xh": xh[c * NPAIR:(c + 1) * NPAIR],
        })

    nc = _get_state()
    res = run_bass_kernel_spmd(nc, in_maps, core_ids=list(range(NCORES)))

    bias = C["bint"][:MD]
    bl = (C["bl"] - C["bint"])[:MD]
    br = (C["br"] - C["bint"])[:MD]
    out = np.empty((B, T, COUT), np.float32)
    for c in range(NCORES):
        o = res.results[c]["out"].astype(np.float32)             # [64, BL, 256]
        out[c * BL:(c + 1) * BL, :, :MD] = o.transpose(1, 2, 0)
    out[:, :, :MD] += bias
    out[:, 0, :MD] += bl
    out[:, T - 1, :MD] += br
    out[:, :, MD:] = h2
    return out.reshape(B, T, J, POS)


def _subproc_main(in_path, out_path):
    with open(in_path, "rb") as f:
        import pickle
        inputs = pickle.load(f)
    np.save(out_path, _kernel_impl(**inputs))


def kernel(**inputs):
    """Entry point. The very first execution of a freshly compiled NEFF
    occasionally kills the device session (NRT_EXEC_UNIT_UNRECOVERABLE);
    a rerun in a fresh process reliably succeeds (the compile cache makes
    it cheap). So: try in-process, fall back to fresh subprocesses."""
    if not _STATE.get("dead"):
        try:
            return _kernel_impl(**inputs)
        except Exception:  # noqa: BLE001
            _STATE["dead"] = True  # this process's device session is gone

    import pickle
    import subprocess
    import tempfile

    kdir = os.path.dirname(os.path.abspath(__file__))
    last_err = None
    for _ in range(3):
        with tempfile.TemporaryDirectory() as td:
            ip = os.path.join(td, "in.pkl")
            op = os.path.join(td, "out.npy")
            with open(ip, "wb") as f:
                pickle.dump({k: np.asarray(v) for k, v in inputs.items()}, f,
                            protocol=4)
            code = (
                "import sys; sys.path.insert(0, {kd!r}); import kernel; "
                "kernel._subproc_main({ip!r}, {op!r})"
            ).format(kd=kdir, ip=ip, op=op)
            r = subprocess.run([sys.executable, "-c", code],
                               capture_output=True, text=True)
            if r.returncode == 0 and os.path.exists(op):
                return np.load(op)
            last_err = r.stderr[-2000:] if r.stderr else f"rc={r.returncode}"
    raise RuntimeError(f"kernel subprocess retries exhausted: {last_err}")


# revision 18
# speedup vs baseline: 1.3499x; 1.0561x over previous
"""Trainium2 Bass kernel for nn_Encoder_29661044146233 (gnn_message_passing).

Approach
--------
The network is linear per output frame, so it folds into a single 22-tap
stride-8 conv (88 -> 66 channels) whose weights are probed on the host in
float64 (see _compose).  The composed map out[t] = A xblk[t-1] + B xblk[t]
+ C xblk[t+1] runs on 8-frame input blocks (704 values zero-padded to
768 = 6*128 so the contraction tiles the full 128-partition dim).

This version runs the matmuls in fp8e4 with the DoubleRow perf mode
(2 K-tiles of 128 per instruction at 0.5 cycles/row), which the TRN2 ISA
permits for M <= 64 and 16-byte-aligned weight pair strides.  The device
therefore computes output rows 0..63; the remaining 2 of the 66 channels
are computed exactly on the host (cheap numpy) and stitched in.

fp8 e4m3 alone is ~3.8% off, so the contraction is error-corrected:

    W x ~= Whi xhi  +  Wlo xhi  +  Whi xlo

with Whi = e4m3(W), Wlo = e4m3(W - Whi), xhi = e4m3(x), xlo = e4m3(x-xhi)
(all at global power-of-2 scales to dodge the e4m3 subnormal floor).  The
two correction terms are band-limited to the high-energy taps (the exact
chunk pairs already present in the main term), which measures ~0.8%
end-to-end vs the 2e-2 tolerance.  Per batch: 9 DR (main) + 7 DR (W corr)
+ 7 DR (x corr) accumulating into one PSUM tile, plus N=1 fp16 edge-delta
matmuls for the boundary columns.  The x-corr reuses the main term's
weight slots, so the weight blob stays small.

Inputs go to the device as two stacked fp8 block arrays (hi, lo) per
batch; out is fp16 (scaled back by 2^-15 during the PSUM->SBUF copy).
"""

import os
import sys

for _p in ("/opt/trn_rl_repo", "/root/.axon_site/_ro/trn_rl_repo"):
    if os.path.isdir(_p) and _p not in sys.path:
        sys.path.append(_p)

import numpy as np
import ml_dtypes

TOPOLOGY = [0, 0, 1, 2, 3, 4, 0, 6, 7, 8, 0, 10, 11, 12, 12, 14, 15, 16, 12, 18, 19, 20]
J = 22
POS, OFF = 3, 1
CIN = 88
COUT = 66
MD = 64                   # device-computed output rows (DoubleRow M cap)
NTAP = 22
NEDGE = 15
B, F, T = 128, 2048, 256
NCORES = 8
BL = B // NCORES          # batch per core
NPAIR = BL // 2
UB = 258                  # blocks incl one zero pad each side
BK = 768                  # padded block length (704 data + 64 zero pad)
KC = 6                    # K chunks of 128 per block
XC = UB * KC              # sbuf cols per batch per array (hi or lo)
SW = 2.0 ** 10            # weight scale before e4m3
SX = 2.0 ** 5             # input scale before e4m3
SOUT = 1.0 / (SW * SX)

# DoubleRow slot tables: (slot, u0, c) with u0 the rhs block-window (0=A/x[t-1],
# 1=B/x[t], 2=C/x[t+1]) and c the even base chunk of the (c, c+1) pair.
# slot 8 pairs C-chunk4 with zero weights (C has 5 nonzero chunks).
MAIN = [(0, 1, 0), (1, 1, 2), (2, 1, 4),
        (3, 0, 0), (4, 0, 2), (5, 0, 4),
        (6, 2, 0), (7, 2, 2), (8, 2, 4)]
WCORR = [(9, 1, 0), (10, 1, 2), (11, 1, 4),
         (12, 0, 2), (13, 0, 4),
         (14, 2, 0)]
XCORR = [(0, 1, 0), (1, 1, 2), (2, 1, 4),
         (4, 0, 2), (5, 0, 4),
         (6, 2, 0)]
NSLOT = 15
ND = 3                    # edge-delta frames per side
CE_W = 2 * ND * MD
CE_X = 2 * ND * BL


# ---------------------------------------------------------------------------
# host-side weight composition (float64 impulse probing) — unchanged
# ---------------------------------------------------------------------------

def _adj():
    a = np.zeros((J, J), np.float64)
    for i, p in enumerate(TOPOLOGY):
        if i:
            a[p, i] = 1.0
    return a


def _conv_np(z, w, b):
    Bn, Fn, C = z.shape
    zp = np.zeros((Bn, Fn + 2, C), z.dtype)
    zp[:, 1:Fn + 1] = z
    Fo = Fn // 2
    out = np.zeros((Bn, Fo, w.shape[0]), z.dtype)
    for k in range(4):
        out += zp[:, k:k + 2 * Fo:2] @ w[:, :, k].T
    return out + b


def _graph_mat(A, n2n_w, n2n_b, e2n_we, e2n_wn, e2n_b,
               n2e_wn, n2e_we, n2e_b, lin_w, lin_b):
    def apply(z):
        sh = z.shape[:-1]
        zz = z.reshape(-1, J, 4)
        node, edge = zz[..., :POS], zz[..., POS:]
        agg_n = np.einsum('ij,bjc->bic', A, node)
        agg_e = np.einsum('ij,bjc->bic', A, edge)
        f1 = agg_n @ n2n_w + n2n_b
        f2 = agg_e @ e2n_we + node @ e2n_wn + e2n_b
        new_edge = (np.einsum('ji,bjc->bic', A, node) @ n2e_wn
                    + edge @ n2e_we + n2e_b)
        h = np.concatenate([f1, f2], axis=-1) @ lin_w + lin_b
        return np.concatenate([h, new_edge], axis=-1).reshape(*sh, 88)

    g = apply(np.zeros((1, 88)))[0]
    G = apply(np.eye(88)) - g
    return G.T, g


def _compose(P):
    A = _adj()
    P64 = {k: np.asarray(v, np.float64) for k, v in P.items()}
    gnames = ('n2n_w', 'n2n_b', 'e2n_we', 'e2n_wn', 'e2n_b',
              'n2e_wn', 'n2e_we', 'n2e_b', 'lin_w', 'lin_b')
    G1, g1 = _graph_mat(A, *[P64['g1_' + s] for s in gnames])
    G2, g2 = _graph_mat(A, *[P64['g2_' + s] for s in gnames])
    keep = np.array([4 * j + c for j in range(J) for c in range(POS)])

    def pipeline(x88):
        y = _conv_np(x88, P64['conv1_w'], P64['conv1_b'])
        y = y @ G1.T + g1
        y = _conv_np(y, P64['conv2_w'], P64['conv2_b'])
        y = y @ G2.T + g2
        y = _conv_np(y, P64['conv3_w'], P64['conv3_b'])
        return y[..., keep]

    Fp = 256
    Tp = Fp // 8
    zb = pipeline(np.zeros((1, Fp, 88)))[0]
    bint, bl, br = zb[Tp // 2], zb[0], zb[Tp - 1]

    mid = Fp // 2
    probes = np.zeros((8 * 88, Fp, 88))
    for r in range(8):
        for ic in range(88):
            probes[r * 88 + ic, mid + r, ic] = 1.0
    resp = pipeline(probes) - zb
    wint = np.zeros((NTAP, COUT, CIN))
    for r in range(8):
        for t in range(Tp):
            m = (mid + r) - 8 * t + 7
            if 0 <= m < NTAP:
                wint[m] = resp[r * 88:(r + 1) * 88, t, :].T

    probes = np.zeros((NEDGE * 88, Fp, 88))
    for f in range(NEDGE):
        for ic in range(88):
            probes[f * 88 + ic, f, ic] = 1.0
    resp = pipeline(probes) - zb
    wl = np.stack([resp[f * 88:(f + 1) * 88, 0, :].T for f in range(NEDGE)])

    probes = np.zeros((NEDGE * 88, Fp, 88))
    for f in range(NEDGE):
        for ic in range(88):
            probes[f * 88 + ic, Fp - NEDGE + f, ic] = 1.0
    resp = pipeline(probes) - zb
    wr = np.stack([resp[f * 88:(f + 1) * 88, Tp - 1, :].T for f in range(NEDGE)])

    return dict(wint=wint, bint=bint, wl=wl, wr=wr, bl=bl, br=br)


# ---------------------------------------------------------------------------
# device program (built/compiled once, reused across calls)
# ---------------------------------------------------------------------------

_STATE = {}

DEFAULT_OPTS = dict(
    warm_n=18,          # warm-up matmul count (bridge p-state ramp)
    memset_cols=2,
    xs_bufs=5,
    tail_cols=64,       # final col-chain width (T = no split)
)


def _build_device(opts=None):
    import concourse.bass as bass  # noqa: F401
    import concourse.tile as tile
    from concourse import bacc, mybir

    o_ = dict(DEFAULT_OPTS)
    if opts:
        o_.update(opts)
    f32 = mybir.dt.float32
    f16 = mybir.dt.float16
    f8 = mybir.dt.float8e4
    DR = mybir.MatmulPerfMode.DoubleRow
    nc = bacc.Bacc("TRN2", target_bir_lowering=False, debug=False,
                   num_devices=NCORES)

    wsb_d = nc.dram_tensor("wsb", [128, NSLOT * 128], f8, kind="ExternalInput")
    we_d = nc.dram_tensor("we", [CIN, CE_W + CE_X], f16, kind="ExternalInput")
    xh_d = nc.dram_tensor("xh", [NPAIR, 128, 2, 2 * XC], f8, kind="ExternalInput")
    out_d = nc.dram_tensor("out", [MD, BL, T], f16, kind="ExternalOutput")

    with tile.TileContext(nc) as tc:
        with (
            tc.tile_pool(name="consts", bufs=1) as consts,
            tc.tile_pool(name="xs", bufs=o_["xs_bufs"]) as xspool,
            tc.tile_pool(name="ps1", bufs=4, space="PSUM") as ps1pool,
            tc.tile_pool(name="warm", bufs=1, space="PSUM") as warmpool,
            tc.tile_pool(name="ob", bufs=1) as opool,
        ):
            # PE warm-up: dummy bf16 matmuls on scratch, no DMA deps, to
            # bridge the ~3us p-state ramp while the first DMAs stream.
            bf16 = mybir.dt.bfloat16
            scratch = consts.tile([CIN, 162], f32)
            if o_["memset_cols"]:
                nc.vector.memset(scratch[:, 0:o_["memset_cols"]], 0.0)
            s16 = scratch[:].bitcast(bf16)
            wps = warmpool.tile([COUT, 256], f32)
            for _ in range(o_["warm_n"]):
                nc.tensor.matmul(wps[:], lhsT=s16[:, 0:COUT],
                                 rhs=s16[:, 66:322], start=True, stop=True)

            # DMA order: weight blob + batch-0 input first, then edge blob,
            # then per-pair input streams.
            wsb = consts.tile([128, NSLOT, 2, MD], f8)
            x0 = xspool.tile([128, 2, 2, XC], f8)
            nc.scalar.dma_start(out=wsb[:], in_=wsb_d[:])
            nc.sync.dma_start(out=x0[:, 0], in_=xh_d[0][:, 0].rearrange(
                "p (a x) -> p a x", a=2))
            we_sb_t = consts.tile([CIN, CE_W + CE_X], f16)
            nc.scalar.dma_start(out=we_sb_t[:], in_=we_d[:])
            nc.sync.dma_start(out=x0[:, 1], in_=xh_d[0][:, 1].rearrange(
                "p (a x) -> p a x", a=2))
            we_sb = we_sb_t[:, 0:CE_W].rearrange(
                "c (s e o) -> c s e o", s=2, e=ND)
            xe_sb = we_sb_t[:, CE_W:CE_W + CE_X].rearrange(
                "c (s e b) -> c s e b", s=2, e=ND)

            def xpair(p):
                # two single-batch DMAs: supply (1101ns/batch) then tracks
                # just ahead of the PE burn rate (~1167ns/batch), no stalls
                xt = xspool.tile([128, 2, 2, XC], f8)
                for b in range(2):
                    nc.sync.dma_start(
                        out=xt[:, b],
                        in_=xh_d[p][:, b].rearrange("p (a x) -> p a x", a=2))
                return xt

            # split output staging into 3 tiles so each store DMA depends
            # only on its own batches (coarse tile deps otherwise park the
            # mid store behind the final batch)
            obA = opool.tile([MD, 7, T], f16)
            obB = opool.tile([MD, 8, T], f16)
            obC = opool.tile([MD, 1, T], f16)

            def obsel(boff):
                if boff < 7:
                    return obA, boff
                if boff < 15:
                    return obB, boff - 7
                return obC, 0

            def conv(xt, b0, boff, c0=0, nc_=T):
                # one batch; out col window [c0, c0+nc_)
                xv = xt[:].rearrange("p b a (u s) -> p b a u s", s=KC)
                t1 = ps1pool.tile([MD, nc_], f32)
                sides = [s for s, on in ((0, c0 == 0), (1, c0 + nc_ == T))
                         if on]
                nmm = len(MAIN) + len(WCORR) + len(XCORR) + ND * len(sides)
                k = 0

                def rhs(a, u0, c):
                    return xv[:, b0, a, u0 + c0:u0 + c0 + nc_, c:c + 2] \
                        .rearrange("p u s -> p s u")

                for table, a in ((MAIN, 0), (WCORR, 0), (XCORR, 1)):
                    for slot, u0, c in table:
                        nc.tensor.matmul(
                            t1[:], lhsT=wsb[:, slot], rhs=rhs(a, u0, c),
                            start=(k == 0), stop=False, perf_mode=DR)
                        k += 1
                for side in sides:
                    col = 0 if side == 0 else nc_ - 1
                    xe = xe_sb[:, side, :, boff:boff + 1].rearrange(
                        "c e (b x) -> c e b x", x=1)
                    for e in range(ND):
                        k += 1
                        nc.tensor.matmul(
                            t1[:, col:col + 1],
                            lhsT=we_sb[:, side, e, :], rhs=xe[:, e],
                            start=False, stop=(k == nmm))

                ot, oi = obsel(boff)
                o = ot[:, oi:oi + 1, c0:c0 + nc_].rearrange("m b n -> m (b n)")
                nc.scalar.activation(
                    o, t1[:], mybir.ActivationFunctionType.Identity,
                    bias=0.0, scale=SOUT)

            x1 = xpair(1)
            conv(x0, 0, 0)
            conv(x0, 1, 1)
            conv(x1, 0, 2)
            conv(x1, 1, 3)
            for p in range(2, NPAIR - 1):
                xt = xpair(p)
                conv(xt, 0, 2 * p)
                conv(xt, 1, 2 * p + 1)
            xl = xpair(NPAIR - 1)
            # obA store issued on the same in-order queue after the last
            # input fetch so its transfer cannot delay those batches
            nc.sync.dma_start(out=out_d[:, 0:7, :], in_=obA[:])
            conv(xl, 0, BL - 2)
            nc.sync.dma_start(out=out_d[:, 7:15, :], in_=obB[:])
            tc_ = o_["tail_cols"]
            if tc_ == T:
                conv(xl, 1, BL - 1)
            else:
                conv(xl, 1, BL - 1, 0, T - tc_)
                conv(xl, 1, BL - 1, T - tc_, tc_)
            nc.sync.dma_start(out=out_d[:, 15:BL, :], in_=obC[:])

    nc.compile()
    return nc


def _get_state():
    if "nc" not in _STATE:
        _STATE["nc"] = _build_device()
    return _STATE["nc"]


# ---------------------------------------------------------------------------
# host packing
# ---------------------------------------------------------------------------

def _fp8(v):
    return np.asarray(v, dtype=ml_dtypes.float8_e4m3fn)


def _host_pack(C, x88):
    """Marshal composed weights + inputs into the device tensors."""
    wint = C["wint"]

    Am = np.zeros((COUT, BK))
    Bm = np.zeros((COUT, BK))
    Cm = np.zeros((COUT, BK))
    for m in range(NTAP):
        if m < 7:
            Am[:, 88 * (m + 1):88 * (m + 2)] = wint[m]
        elif m < 15:
            Bm[:, 88 * (m - 7):88 * (m - 6)] = wint[m]
        else:
            Cm[:, 88 * (m - 15):88 * (m - 14)] = wint[m]
    maps = {0: Am, 1: Bm, 2: Cm}
    hi = {}
    lo = {}
    for u0, M in maps.items():
        h = _fp8(M * SW)
        hi[u0] = h
        lo[u0] = _fp8(M * SW - h.astype(np.float64))

    wsb = np.zeros((128, NSLOT, 2, MD), ml_dtypes.float8_e4m3fn)
    for slot, u0, c in MAIN:
        for j in range(2):
            cc = c + j
            if cc < KC and not (u0 == 2 and cc == 5):
                wsb[:, slot, j, :] = hi[u0][:MD, 128 * cc:128 * cc + 128].T
    for slot, u0, c in WCORR:
        for j in range(2):
            cc = c + j
            if cc < KC and not (u0 == 2 and cc == 5):
                wsb[:, slot, j, :] = lo[u0][:MD, 128 * cc:128 * cc + 128].T
    wsb = wsb.reshape(128, NSLOT * 128)

    # input marshalling: [B, F, 88] -> scaled hi/lo padded blocks
    xb = np.zeros((B, UB, BK))
    xb[:, 1:257, :704] = x88.reshape(B, T, 704) * SX
    xhi = _fp8(xb)
    xlo = _fp8(xb - xhi.astype(np.float64))
    xs = np.stack([xhi, xlo], axis=1)        # [B, 2, UB, BK]
    xh = np.ascontiguousarray(
        xs.reshape(B // 2, 2, 2, UB, KC, 128).transpose(0, 5, 1, 2, 3, 4)
    ).reshape(B // 2, 128, 2, 2 * XC)

    # edge delta weights/inputs (fp16, scaled to match the fp8 terms)
    x88T = x88.transpose(0, 2, 1)                                # [B, 88, F]
    xedge = np.zeros((B, CIN, 2, ND), np.float16)
    xedge[:, :, 0, :] = x88T[:, :, :ND] * SX
    xedge[:, :, 1, :] = x88T[:, :, F - ND:] * SX

    dwl = (C["wl"][:3] - wint[7:10]).transpose(2, 0, 1)          # [88, 3, 66]
    dwr = (C["wr"][12:15] - wint[12:15]).transpose(2, 0, 1)
    wedge = np.zeros((CIN, 2, ND, MD), np.float16)
    wedge[:, 0, :, :] = dwl[:, :, :MD] * SW
    wedge[:, 1, :, :] = dwr[:, :, :MD] * SW
    return wsb, wedge, xedge, xh


def _core_we(wedge, xedge, c):
    s = slice(c * BL, (c + 1) * BL)
    return np.concatenate([
        wedge.reshape(CIN, -1),
        np.ascontiguousarray(
            xedge[s].transpose(1, 2, 3, 0)).reshape(CIN, -1),
    ], axis=1)


def _host_tail(C, x88):
    """Exact host computation of output channels MD..66 plus the bias
    terms (all-zero for the given inputs, kept for generality)."""
    wint, wl, wr = C["wint"], C["wl"], C["wr"]
    xp = np.zeros((B, F + 16, CIN))
    xp[:, 7:7 + F] = x88
    h2 = np.zeros((B, T, COUT - MD))
    for m in range(NTAP):
        h2 += xp[:, m:m + 8 * T:8] @ wint[m, MD:COUT].T
    dwl = wl[:3] - wint[7:10]
    dwr = wr[12:15] - wint[12:15]
    for e in range(ND):
        h2[:, 0] += x88[:, e] @ dwl[e, MD:COUT].T
        h2[:, T - 1] += x88[:, F - ND + e] @ dwr[e, MD:COUT].T
    h2 += C["bint"][MD:COUT]
    h2[:, 0] += (C["bl"] - C["bint"])[MD:COUT]
    h2[:, T - 1] += (C["br"] - C["bint"])[MD:COUT]
    return h2


# ---------------------------------------------------------------------------
# entry point
# ---------------------------------------------------------------------------

def _kernel_impl(**inputs):
    from concourse.bass_utils import run_bass_kernel_spmd

    P = {k: np.asarray(v) for k, v in inputs.items()}
    inp = P.pop("input").astype(np.float64, copy=False)
    off = P.pop("offset").astype(np.float64, copy=False)
    x88 = np.concatenate([inp, off], -1).reshape(B, F, CIN)

    C = _compose(P)
    wsb, wedge, xedge, xh = _host_pack(C, x88)
    h2 = _host_tail(C, x88)

    in_maps = []
    for c in range(NCORES):
        in_maps.append({
            "wsb": wsb,
            "we": _core_we(wedge, xedge, c),
            "xh": xh[c * NPAIR:(c + 1) * NPAIR],
        })

    nc = _get_state()
    res = run_bass_kernel_spmd(nc, in_maps, core_ids=list(range(NCORES)))

    bias = C["bint"][:MD]
    bl = (C["bl"] - C["bint"])[:MD]
    br = (C["br"] - C["bint"])[:MD]
    out = np.empty((B, T, COUT), np.float32)
    for c in range(NCORES):
        o = res.results[c]["out"].astype(np.float32)             # [64, BL, 256]
        out[c * BL:(c + 1) * BL, :, :MD] = o.transpose(1, 2, 0)
    out[:, :, :MD] += bias
    out[:, 0, :MD] += bl
    out[:, T - 1, :MD] += br
    out[:, :, MD:] = h2
    return out.reshape(B, T, J, POS)


def _subproc_main(in_path, out_path):
    with open(in_path, "rb") as f:
        import pickle
        inputs = pickle.load(f)
    np.save(out_path, _kernel_impl(**inputs))


def kernel(**inputs):
    """Entry point. The very first execution of a freshly compiled NEFF
    occasionally kills the device session (NRT_EXEC_UNIT_UNRECOVERABLE);
    a rerun in a fresh process reliably succeeds (the compile cache makes
    it cheap). So: try in-process, fall back to fresh subprocesses."""
    if not _STATE.get("dead"):
        try:
            return _kernel_impl(**inputs)
        except Exception:  # noqa: BLE001
            _STATE["dead"] = True  # this process's device session is gone

    import pickle
    import subprocess
    import tempfile

    kdir = os.path.dirname(os.path.abspath(__file__))
    last_err = None
    for _ in range(3):
        with tempfile.TemporaryDirectory() as td:
            ip = os.path.join(td, "in.pkl")
            op = os.path.join(td, "out.npy")
            with open(ip, "wb") as f:
                pickle.dump({k: np.asarray(v) for k, v in inputs.items()}, f,
                            protocol=4)
            code = (
                "import sys; sys.path.insert(0, {kd!r}); import kernel; "
                "kernel._subproc_main({ip!r}, {op!r})"
            ).format(kd=kdir, ip=ip, op=op)
            r = subprocess.run([sys.executable, "-c", code],
                               capture_output=True, text=True)
            if r.returncode == 0 and os.path.exists(op):
                return np.load(op)
            last_err = r.stderr[-2000:] if r.stderr else f"rc={r.returncode}"
    raise RuntimeError(f"kernel subprocess retries exhausted: {last_err}")
